# revision 9
# baseline (speedup 1.0000x reference)
"""Cross-Mamba-Attention fused Trainium2 kernel (8-core SPMD).

Shapes (hardcoded): B=4, L=4096, D=256, DI=512, DS=16, DR=16, DC=4.
Sharding: core c -> (batch b = c//2, DI-half dh = c%2). Each core computes
attention + mamba for its batch, scans its DI-half, and emits a partial
(256, 4096) channel-major output; host sums core pairs and transposes.

All per-core variation is input-driven (host permutes the DI axis of the
weights so the scanned half is always channels 0:256) -> one SPMD program.
"""
import os
import numpy as np
import ml_dtypes

import concourse.bass as bass
import concourse.bacc as bacc
import concourse.mybir as mybir
import concourse.tile as tile
from concourse import bass_utils

F32 = mybir.dt.float32
F32R = mybir.dt.float32r
BF16 = mybir.dt.bfloat16
AF = mybir.ActivationFunctionType
OP = mybir.AluOpType
AX = mybir.AxisListType

B, L, D = 4, 4096, 256
DI, DS, DR, DC = 512, 16, 16, 4
SCALE = float(D) ** -0.5
EPS = 1e-5
NC8 = 8
LC = L // 512          # 8 n-chunks of 512
LT = L // 128          # 32 l-tiles of 128

_BF = ml_dtypes.bfloat16


def _r(x):
    return x.bitcast(F32R)


def build_program(debug=False):
    nc = bacc.Bacc("TRN2", target_bir_lowering=False, debug=False,
                   enable_asserts=False, num_devices=NC8)

    # ---- dram I/O ----
    d_xT = nc.dram_tensor("xT", [D, L], F32, kind="ExternalInput").ap()
    d_cT = nc.dram_tensor("cT", [D, L], F32, kind="ExternalInput").ap()
    d_wq = nc.dram_tensor("wq", [128, 2, D], F32, kind="ExternalInput").ap()
    d_wkv = nc.dram_tensor("wkv", [128, 2, 2 * D], F32, kind="ExternalInput").ap()
    d_lnwb = nc.dram_tensor("lnwb", [128, 2, 2], F32, kind="ExternalInput").ap()
    d_wcv = nc.dram_tensor("wcv", [128, 2, DC, DI], F32, kind="ExternalInput").ap()
    d_cb = nc.dram_tensor("cb", [128, 4], F32, kind="ExternalInput").ap()
    d_wz = nc.dram_tensor("wz", [128, 2, D], F32, kind="ExternalInput").ap()
    d_xpw = nc.dram_tensor("xpw", [128, 4, 48], BF16, kind="ExternalInput").ap()
    d_wdt = nc.dram_tensor("wdt", [DR, D], BF16, kind="ExternalInput").ap()
    d_dtb = nc.dram_tensor("dtb", [128, 2], F32, kind="ExternalInput").ap()
    d_A = nc.dram_tensor("Aneg", [128, 2, DS], F32, kind="ExternalInput").ap()
    d_dsk = nc.dram_tensor("dsk", [128, 2], F32, kind="ExternalInput").ap()
    d_pw = nc.dram_tensor("pw", [128, 2, D], BF16, kind="ExternalInput").ap()
    d_out = nc.dram_tensor("outT", [D, L], F32, kind="ExternalOutput").ap()
    dbg = {}
    if debug:
        dbg["attn"] = nc.dram_tensor("dbg_attn", [128, 2, D], BF16, kind="ExternalOutput").ap()
        dbg["sim"] = nc.dram_tensor("dbg_sim", [128, 2, D], F32, kind="ExternalOutput").ap()
        dbg["ex"] = nc.dram_tensor("dbg_ex", [128, 2, D], F32, kind="ExternalOutput").ap()
        dbg["mx"] = nc.dram_tensor("dbg_mx", [128, 2], F32, kind="ExternalOutput").ap()
        dbg["sm"] = nc.dram_tensor("dbg_sm", [128, 2], F32, kind="ExternalOutput").ap()
        dbg["qt"] = nc.dram_tensor("dbg_qt", [128, 2, L], BF16, kind="ExternalOutput").ap()
        dbg["ob"] = nc.dram_tensor("dbg_ob", [128, 2, L], F32, kind="ExternalOutput").ap()
        dbg["var"] = nc.dram_tensor("dbg_var", [1, L], F32, kind="ExternalOutput").ap()
        dbg["hn"] = nc.dram_tensor("dbg_hn", [128, 2, L + 3], F32, kind="ExternalOutput").ap()
        dbg["xso"] = nc.dram_tensor("dbg_xso", [128, 2, L], BF16, kind="ExternalOutput").ap()
        dbg["bc"] = nc.dram_tensor("dbg_bc", [32, L], BF16, kind="ExternalOutput").ap()
        dbg["dt"] = nc.dram_tensor("dbg_dt", [128, 2, L], BF16, kind="ExternalOutput").ap()
        dbg["y"] = nc.dram_tensor("dbg_y", [128, 2, L], BF16, kind="ExternalOutput").ap()
        dbg["sz"] = nc.dram_tensor("dbg_sz", [128, 2, L], BF16, kind="ExternalOutput").ap()

    with tile.TileContext(nc) as tc:
        with (
            tc.tile_pool(name="const", bufs=1) as cst,
            tc.tile_pool(name="long", bufs=1) as lng,
            tc.tile_pool(name="dram", bufs=1, space="DRAM") as drm,
            tc.tile_pool(name="psum", bufs=1, space="PSUM") as ps,
        ):
            # ---------- constants (~27 KB/part) ----------
            wq_sb = cst.tile([128, 2, D], F32R)
            nc.gpsimd.dma_start(wq_sb[:], d_wq)
            wkv_sb = cst.tile([128, 2, 2 * D], F32R)
            nc.gpsimd.dma_start(wkv_sb[:], d_wkv)
            wcv_sb = cst.tile([128, 2, DC, DI], F32R)
            nc.gpsimd.dma_start(wcv_sb[:], d_wcv)
            wz_sb = cst.tile([128, 2, D], F32R)
            nc.gpsimd.dma_start(wz_sb[:], d_wz)
            xpw_sb = cst.tile([128, 4, 48], BF16)
            nc.sync.dma_start(xpw_sb[:], d_xpw)
            wdt_sb = cst.tile([DR, D], BF16)
            nc.sync.dma_start(wdt_sb[:], d_wdt)
            pw_sb = cst.tile([128, 2, D], BF16)
            nc.sync.dma_start(pw_sb[:], d_pw)
            lnwb_sb = cst.tile([128, 2, 2], F32)   # [p, a, {w,b}]
            nc.sync.dma_start(lnwb_sb[:], d_lnwb)
            cb_sb = cst.tile([128, 4], F32)
            nc.sync.dma_start(cb_sb[:], d_cb)
            dtb_sb = cst.tile([128, 2], F32)
            nc.sync.dma_start(dtb_sb[:], d_dtb)
            A_sb = cst.tile([128, 2, DS], F32)
            nc.sync.dma_start(A_sb[:], d_A)
            dsk_sb = cst.tile([128, 2], F32)
            nc.sync.dma_start(dsk_sb[:], d_dsk)
            ones_col = cst.tile([128, 1], F32)
            nc.vector.memset(ones_col[:], 1.0)
            ones_r = cst.tile([128, 1], F32R)
            nc.vector.tensor_copy(ones_r[:], ones_col[:])
            zero_col = cst.tile([128, 1], F32)
            nc.vector.memset(zero_col[:], 0.0)
            eps_col = cst.tile([1, 1], F32)
            nc.vector.memset(eps_col[:], EPS / 4.0)

            # ---------- long-lived ----------
            y_sb = lng.tile([128, 2, L], BF16)          # sum_s C*h accumulator

            bc_dram = drm.tile([32, L], BF16)           # Bm rows 0:16, Cm rows 16:32
            rs_dram = drm.tile([1, L], F32)             # rstd row bounce
            hn_dram = drm.tile([128, 2, L + 3], F32)    # LN output (conv-padded)
            xso_dram = drm.tile([128, 2, L], BF16)      # xs own half spill
            sz_dram = drm.tile([128, 2, L], BF16)       # silu(z) spill

            # =============== S1: attention -> hn (to DRAM) ===============
            with tc.tile_pool(name="s1", bufs=1) as s1:
                ct_sb = s1.tile([128, 2, L], F32R, tag="bigin")
                nc.gpsimd.dma_start(ct_sb[:], d_cT.rearrange("(a p) l -> p a l", p=128))

                sim_ps0 = ps.tile([128, D], F32, tag="sim", bufs=2)
                sim_ps1 = ps.tile([128, D], F32, tag="sim", bufs=2)
                sim_psl = [sim_ps0, sim_ps1]
                for lt in range(LT):
                    kv_ps = ps.tile([128, 2 * D], F32, tag="mm", bufs=3)
                    for a in range(2):
                        nc.tensor.matmul(kv_ps[:], ct_sb[:, a, lt * 128:(lt + 1) * 128],
                                         wkv_sb[:, a, :], start=(a == 0), stop=(a == 1))
                    kv_sb = s1.tile([128, 2 * D], F32R, tag="kvsb", bufs=2)
                    nc.scalar.copy(kv_sb[:], kv_ps[:])
                    for mt in range(2):
                        nc.tensor.matmul(sim_psl[mt][:],
                                         kv_sb[:, mt * 128:(mt + 1) * 128],
                                         kv_sb[:, D:2 * D],
                                         start=(lt == 0), stop=(lt == LT - 1))

                # softmax over free dim e, centered: attn = softmax - 1/D
                attn_sb = s1.tile([128, 2, D], BF16)
                if debug:
                    sim_cp = s1.tile([128, 2, D], F32)
                    nc.scalar.copy(sim_cp[:, 0, :], sim_ps0[:])
                    nc.scalar.copy(sim_cp[:, 1, :], sim_ps1[:])
                    nc.sync.dma_start(dbg["sim"], sim_cp[:])
                    ex_cp = s1.tile([128, 2, D], F32)
                    mx_cp = s1.tile([128, 2], F32)
                    sm_cp = s1.tile([128, 2], F32)
                for mt in range(2):
                    mx = s1.tile([128, 1], F32, tag="mx", bufs=2)
                    nc.vector.tensor_reduce(mx[:], sim_psl[mt][:], AX.X, OP.max)
                    mxs = s1.tile([128, 1], F32, tag="mxs", bufs=2)
                    nc.vector.tensor_scalar(mxs[:], mx[:], -SCALE, None, OP.mult)
                    ex = s1.tile([128, D], F32, tag="ex", bufs=2)
                    nc.scalar.activation(ex[:], sim_psl[mt][:], AF.Exp,
                                         bias=mxs[:, 0:1], scale=SCALE)
                    sm = s1.tile([128, 1], F32, tag="sm", bufs=2)
                    nc.vector.tensor_reduce(sm[:], ex[:], AX.X, OP.add)
                    rs = s1.tile([128, 1], F32, tag="rs", bufs=2)
                    nc.vector.reciprocal(rs[:], sm[:])
                    nc.vector.tensor_scalar(attn_sb[:, mt, :], ex[:], rs[:, 0:1],
                                            1.0 / D, OP.mult, OP.subtract)
                    if debug:
                        nc.vector.tensor_copy(ex_cp[:, mt, :], ex[:])
                        nc.vector.tensor_copy(mx_cp[:, mt:mt + 1], mx[:])
                        nc.vector.tensor_copy(sm_cp[:, mt:mt + 1], sm[:])
                if debug:
                    nc.sync.dma_start(dbg["ex"], ex_cp[:])
                    nc.sync.dma_start(dbg["mx"], mx_cp[:])
                    nc.sync.dma_start(dbg["sm"], sm_cp[:])
                    nc.sync.dma_start(dbg["attn"], attn_sb[:])

                xt_sb = s1.tile([128, 2, L], F32R, tag="bigin")
                nc.gpsimd.dma_start(xt_sb[:], d_xT.rearrange("(a p) l -> p a l", p=128))

                # qT channel-major (bf16)
                qt_sb = s1.tile([128, 2, L], BF16, tag="qt")
                for mt in range(2):
                    for ncc in range(LC):
                        q_ps = ps.tile([128, 512], F32, tag="mm", bufs=3)
                        for a in range(2):
                            nc.tensor.matmul(q_ps[:],
                                             wq_sb[:, a, mt * 128:(mt + 1) * 128],
                                             xt_sb[:, a, ncc * 512:(ncc + 1) * 512],
                                             start=(a == 0), stop=(a == 1))
                        nc.scalar.copy(qt_sb[:, mt, ncc * 512:(ncc + 1) * 512], q_ps[:])

                if debug:
                    nc.sync.dma_start(dbg["qt"], qt_sb[:])
                # obT[e, l] = sum_d attn[d, e] * qT[d, l]  (+ sumsq accumulation)
                ob_sb = s1.tile([128, 2, L], F32, tag="ob")
                var_row = s1.tile([1, L], F32)
                for ncc in range(LC):
                    ssq_ps = ps.tile([1, 512], F32, tag="ssq", bufs=2)
                    for mt in range(2):
                        ob_ps = ps.tile([128, 512], F32, tag="mm", bufs=3)
                        for a in range(2):
                            nc.tensor.matmul(ob_ps[:],
                                             attn_sb[:, a, mt * 128:(mt + 1) * 128],
                                             qt_sb[:, a, ncc * 512:(ncc + 1) * 512],
                                             start=(a == 0), stop=(a == 1))
                        nc.scalar.copy(ob_sb[:, mt, ncc * 512:(ncc + 1) * 512], ob_ps[:])
                        sq = s1.tile([128, 512], F32R, tag="sq", bufs=2)
                        nc.scalar.activation(sq[:], ob_ps[:], AF.Square,
                                             bias=zero_col[:, 0:1])
                        nc.tensor.matmul(ssq_ps[:], ones_r[:], sq[:],
                                         start=(mt == 0), stop=(mt == 1))
                    nc.scalar.copy(var_row[0:1, ncc * 512:(ncc + 1) * 512], ssq_ps[:])

                if debug:
                    nc.sync.dma_start(dbg["ob"], ob_sb[:])
                    nc.sync.dma_start(dbg["var"], var_row[:])
                # rstd = 1/sqrt(sumsq/D + EPS/4), in place on var_row
                nc.scalar.activation(var_row[:], var_row[:], AF.Sqrt,
                                     bias=eps_col[:, 0:1], scale=1.0 / D)
                nc.vector.reciprocal(var_row[:], var_row[:])
                nc.sync.dma_start(rs_dram[:], var_row[:])
                rstd_rep = s1.tile([128, L], F32)
                nc.sync.dma_start(rstd_rep[:], rs_dram[0:1, :].partition_broadcast(128))

                # hn = (ob * rstdRep) * ln_w + ln_b  -> DRAM (padded left by 3)
                zero3 = s1.tile([128, 2, 3], F32)
                nc.vector.memset(zero3[:], 0.0)
                nc.sync.dma_start(hn_dram[:, :, 0:3], zero3[:])
                for a in range(2):
                    hnw = s1.tile([128, L], F32, tag="hnw", bufs=1)
                    nc.vector.tensor_tensor(hnw[:], ob_sb[:, a, :], rstd_rep[:], OP.mult)
                    nc.vector.tensor_scalar(hnw[:], hnw[:],
                                            lnwb_sb[:, a, 0:1], lnwb_sb[:, a, 1:2],
                                            OP.mult, OP.add)
                    nc.sync.dma_start(hn_dram[:, a, 3:], hnw[:])

            if debug:
                nc.sync.dma_start(dbg["hn"], hn_dram[:])
            # ====== sc pool: dt / dtxs (written S2, read S3) ======
            with tc.tile_pool(name="sc", bufs=1) as sc:
                dt_sb = sc.tile([128, 2, L], BF16)
                dtxs_sb = sc.tile([128, 2, L], BF16)

                # =============== S2: mamba projections ===============
                with tc.tile_pool(name="s2", bufs=1) as s2:
                    hn_sb = s2.tile([128, 2, L + 3], F32R)
                    nc.gpsimd.dma_start(hn_sb[:], hn_dram[:])
                    xso_sb = s2.tile([128, 2, L], BF16)
                    xst_sb = s2.tile([128, 2, L], BF16)
                    sz_sb = s2.tile([128, 2, L], BF16)

                    # fused in_proj+conv -> silu -> xs (m-tiles 0,1 own / 2,3 other)
                    for mt in range(4):
                        dst = xso_sb if mt < 2 else xst_sb
                        for ncc in range(LC):
                            xc_ps = ps.tile([128, 512], F32, tag="mm", bufs=3)
                            k = 0
                            for a in range(2):
                                for j in range(DC):
                                    nc.tensor.matmul(
                                        xc_ps[:],
                                        wcv_sb[:, a, j, mt * 128:(mt + 1) * 128],
                                        hn_sb[:, a, ncc * 512 + j: ncc * 512 + j + 512],
                                        start=(k == 0), stop=(k == 2 * DC - 1))
                                    k += 1
                            nc.scalar.activation(dst[:, mt % 2, ncc * 512:(ncc + 1) * 512],
                                                 xc_ps[:], AF.Silu, bias=cb_sb[:, mt:mt + 1])
                    # z -> silu(z) (own half only)
                    for mt in range(2):
                        for ncc in range(LC):
                            z_ps = ps.tile([128, 512], F32, tag="mm", bufs=3)
                            for a in range(2):
                                nc.tensor.matmul(z_ps[:],
                                                 wz_sb[:, a, mt * 128:(mt + 1) * 128],
                                                 hn_sb[:, a, ncc * 512 + 3:(ncc + 1) * 512 + 3],
                                                 start=(a == 0), stop=(a == 1))
                            nc.scalar.activation(sz_sb[:, mt, ncc * 512:(ncc + 1) * 512],
                                                 z_ps[:], AF.Silu, bias=zero_col[:, 0:1])
                    nc.sync.dma_start(sz_dram[:], sz_sb[:])

                    # x_proj: dbl[r, l] = sum_c xs[c, l] * xpw[c, r]
                    dtr_sb = s2.tile([DR, L], BF16)
                    bc_sb = s2.tile([32, L], BF16)
                    for ncc in range(LC):
                        dbl_ps = ps.tile([48, 512], F32, tag="mm", bufs=3)
                        for kc in range(4):
                            src = xso_sb if kc < 2 else xst_sb
                            nc.tensor.matmul(dbl_ps[:], xpw_sb[:, kc, :],
                                             src[:, kc % 2, ncc * 512:(ncc + 1) * 512],
                                             start=(kc == 0), stop=(kc == 3))
                        nc.scalar.copy(bc_sb[:, ncc * 512:(ncc + 1) * 512], dbl_ps[0:32, :])
                        nc.scalar.copy(dtr_sb[:, ncc * 512:(ncc + 1) * 512], dbl_ps[32:48, :])
                    nc.sync.dma_start(bc_dram[:], bc_sb[:])

                    # dt = softplus(dtr @ wdt + dtb) = Ln(Exp(x + dtb) + 1)
                    for mt in range(2):
                        for ncc in range(LC):
                            dt_ps = ps.tile([128, 512], F32, tag="mm", bufs=3)
                            nc.tensor.matmul(dt_ps[:], wdt_sb[:, mt * 128:(mt + 1) * 128],
                                             dtr_sb[:, ncc * 512:(ncc + 1) * 512],
                                             start=True, stop=True)
                            et = s2.tile([128, 512], F32, tag="et", bufs=2)
                            nc.scalar.activation(et[:], dt_ps[:], AF.Exp,
                                                 bias=dtb_sb[:, mt:mt + 1])
                            nc.scalar.activation(dt_sb[:, mt, ncc * 512:(ncc + 1) * 512],
                                                 et[:], AF.Ln, bias=ones_col[:, 0:1])
                    # dtxs = dt * xs(own); then spill xs
                    for a in range(2):
                        nc.vector.tensor_tensor(dtxs_sb[:, a, :], dt_sb[:, a, :],
                                                xso_sb[:, a, :], OP.mult)
                    nc.sync.dma_start(xso_dram[:], xso_sb[:])
                    if debug:
                        nc.sync.dma_start(dbg["xso"], xso_sb[:])
                        nc.sync.dma_start(dbg["bc"], bc_sb[:])
                        nc.sync.dma_start(dbg["dt"], dt_sb[:])
                        nc.sync.dma_start(dbg["sz"], sz_sb[:])

                # =============== S3: selective scan ===============
                with tc.tile_pool(name="s3", bufs=1) as s3:
                    for s in range(DS):
                        bmrep = s3.tile([128, L], BF16, tag="bmrep", bufs=2)
                        nc.sync.dma_start(bmrep[:],
                                          bc_dram[s:s + 1, :].partition_broadcast(128))
                        cmrep = s3.tile([128, L], BF16, tag="cmrep", bufs=2)
                        nc.sync.dma_start(cmrep[:],
                                          bc_dram[DS + s:DS + s + 1, :].partition_broadcast(128))
                        for a in range(2):
                            dA = s3.tile([128, L], BF16, tag="dA", bufs=2)
                            nc.scalar.activation(dA[:], dt_sb[:, a, :], AF.Exp,
                                                 bias=zero_col[:, 0:1],
                                                 scale=A_sb[:, a, s:s + 1])
                            dBu = s3.tile([128, L], BF16, tag="dBu", bufs=2)
                            nc.vector.tensor_tensor(dBu[:], dtxs_sb[:, a, :], bmrep[:],
                                                    OP.mult)
                            h = s3.tile([128, L], BF16, tag="h", bufs=2)
                            nc.vector.tensor_tensor_scan(h[:], dA[:], dBu[:], 0.0,
                                                         OP.mult, OP.add)
                            if s == 0:
                                nc.vector.tensor_tensor(y_sb[:, a, :], h[:], cmrep[:],
                                                        OP.mult)
                            else:
                                hc = s3.tile([128, L], BF16, tag="hc", bufs=2)
                                nc.vector.tensor_tensor(hc[:], h[:], cmrep[:], OP.mult)
                                nc.vector.tensor_tensor(y_sb[:, a, :], y_sb[:, a, :],
                                                        hc[:], OP.add)

            if debug:
                nc.sync.dma_start(dbg["y"], y_sb[:])
            # =============== S4: gate + output projection ===============
            with tc.tile_pool(name="s4", bufs=1) as s4:
                xso2_sb = s4.tile([128, 2, L], BF16)
                nc.sync.dma_start(xso2_sb[:], xso_dram[:])
                sz2_sb = s4.tile([128, 2, L], BF16)
                nc.sync.dma_start(sz2_sb[:], sz_dram[:])
                y2_sb = s4.tile([128, 2, L], BF16)
                for a in range(2):
                    t4 = s4.tile([128, L], BF16, tag="t4", bufs=2)
                    nc.vector.scalar_tensor_tensor(t4[:], xso2_sb[:, a, :],
                                                   dsk_sb[:, a:a + 1], y_sb[:, a, :],
                                                   OP.mult, OP.add)
                    nc.vector.tensor_tensor(y2_sb[:, a, :], t4[:], sz2_sb[:, a, :],
                                            OP.mult)
                out_sb = s4.tile([128, 2, L], F32)
                for mt in range(2):
                    for ncc in range(LC):
                        o_ps = ps.tile([128, 512], F32, tag="mm", bufs=3)
                        for a in range(2):
                            nc.tensor.matmul(o_ps[:], pw_sb[:, a, mt * 128:(mt + 1) * 128],
                                             y2_sb[:, a, ncc * 512:(ncc + 1) * 512],
                                             start=(a == 0), stop=(a == 1))
                        nc.scalar.copy(out_sb[:, mt, ncc * 512:(ncc + 1) * 512], o_ps[:])
                nc.sync.dma_start(d_out.rearrange("(a p) l -> p a l", p=128), out_sb[:])

    nc.compile()
    return nc


_NC = None


def _get_program():
    global _NC
    if _NC is None:
        _NC = build_program(debug=bool(int(os.environ.get("CCK_DEBUG", "0"))))
    return _NC


def kernel(x, context, Wq, Wkv, ln_w, ln_b, in_proj_w, conv_w, conv_b,
           x_proj_w, dt_proj_w, dt_proj_b, A_log, D_skip, out_proj_w, Wout, bout):
    x = np.asarray(x, np.float32)
    context = np.asarray(context, np.float32)
    nc = _get_program()

    P_full = (np.asarray(out_proj_w, np.float64) @ np.asarray(Wout, np.float64)).astype(np.float32)
    A_full = (-np.exp(np.asarray(A_log, np.float64))).astype(np.float32)
    conv_w = np.asarray(conv_w, np.float32)
    in_proj_w = np.asarray(in_proj_w, np.float32)

    in_maps = []
    for c in range(NC8):
        b, dh = c // 2, c % 2
        own = np.arange(dh * D, dh * D + D)
        perm = np.concatenate([own, np.arange((1 - dh) * D, (1 - dh) * D + D)])
        # fused conv+in_proj weights: W_j[m, c'] = ip[m, perm[c']] * cw[perm[c'], j]
        wcv = np.empty((DC, D, DI), np.float32)
        for j in range(DC):
            wcv[j] = in_proj_w[:, :DI][:, perm] * conv_w[perm, j][None, :]

        def pa(w):  # (a*128+p, ...) -> (p, a, ...)
            w = np.asarray(w)
            return np.ascontiguousarray(
                w.reshape(w.shape[0] // 128, 128, *w.shape[1:]).swapaxes(0, 1))

        im = {
            "xT": np.ascontiguousarray(x[b].T),
            "cT": np.ascontiguousarray(context[b].T),
            "wq": pa(np.asarray(Wq, np.float32)),
            "wkv": pa(np.asarray(Wkv, np.float32)),
            "lnwb": pa(np.stack([np.asarray(ln_w, np.float32),
                                 np.asarray(ln_b, np.float32)], axis=-1)),
            "wcv": pa(wcv.transpose(1, 0, 2)),      # (m, j, c) -> (p, a, j, c)
            "cb": pa(np.asarray(conv_b, np.float32)[perm]),
            "wz": pa(np.ascontiguousarray(in_proj_w[:, DI + dh * D: DI + dh * D + D])),
            "xpw": pa(np.asarray(x_proj_w, np.float32)[perm][:, list(range(DR, 48)) + list(range(DR))].astype(_BF)),
            "wdt": np.ascontiguousarray(np.asarray(dt_proj_w, np.float32)[:, own]).astype(_BF),
            "dtb": pa(np.asarray(dt_proj_b, np.float32)[own]),
            "Aneg": pa(np.ascontiguousarray(A_full[own])),
            "dsk": pa(np.asarray(D_skip, np.float32)[own]),
            "pw": pa(np.ascontiguousarray(P_full[own]).astype(_BF)),
        }
        in_maps.append(im)

    trace = bool(int(os.environ.get("CCK_TRACE", "0")))
    res = bass_utils.run_bass_kernel_spmd(nc, in_maps, core_ids=list(range(NC8)),
                                          trace=trace)
    kernel.last_result = res

    out = np.empty((B, L, D), np.float32)
    bout = np.asarray(bout, np.float32)
    for b in range(B):
        part = res.results[2 * b]["outT"] + res.results[2 * b + 1]["outT"]
        out[b] = part.T + bout[None, :]
    return out
